# revision 10
# baseline (speedup 1.0000x reference)
"""DeepWalk hierarchical-softmax scoring kernel for 8 Trainium2 NeuronCores.

Computation (mirrors the nn.Module reference):
    path = heap ancestors of leaf u_k           (L ~ 19-20 static ints)
    emd  = emd_weight[v_j]                      [128]
    hv   = hs_weight[path]                      [L, 128]
    out  = -prod(log_sigmoid(hv @ emd))         scalar f32

Sharding: feature-parallel (column) sharding of both embedding tables —
core c owns dims [16c, 16c+16) of every row. Each core gathers the same
(v_j, path) rows from its own shard, computes partial dots over its 16
dims, and one 8-core AllReduce(add) of the L partial dots reconstructs
the full dots everywhere; the log-sigmoid + product epilogue then runs
replicated and core 0's scalar is returned. This needs a single tiny
collective (row sharding would need two: an emd broadcast AND a
log-prob reduce, since only one core owns row v_j).
"""

import numpy as np

import concourse.bass as bass
import concourse.mybir as mybir
from concourse.bass_utils import run_bass_kernel_spmd

NUM_V = 1_000_000
EMD_DIM = 128
N_CORES = 8
DSH = EMD_DIM // N_CORES  # 16 dims per core
F32 = mybir.dt.float32


def hs_path(u_k: int, num_V: int = NUM_V) -> list[int]:
    """Heap indices of all ancestors of leaf u_k, down-to-root (incl. 0)."""
    n = num_V - 1 + u_k
    path = []
    while n > 0:
        n = (n - 1) // 2
        path.append(n)
    return path


def build_module(v_j: int, u_k: int):
    """Build the per-core Bass module. Lookup indices are compile-time
    constants, mirroring the reference where the path is a static int
    array and v_j/u_k are Python ints."""
    path = hs_path(u_k)
    L = len(path)
    nc = bass.Bass(num_devices=N_CORES)

    emd = nc.dram_tensor("emd", [NUM_V, DSH], F32, kind="ExternalInput")
    hs = nc.dram_tensor("hs", [NUM_V - 1, DSH], F32, kind="ExternalInput")
    out = nc.dram_tensor("out", [1, 1], F32, kind="ExternalOutput")
    cc_in = nc.dram_tensor("cc_in", [1, L], F32)
    cc_out = nc.dram_tensor("cc_out", [1, L], F32, addr_space="Shared")

    n_gather = L + 1  # L row gathers + 1 broadcast emd gather

    with (
        nc.sbuf_tensor("hv", [L, DSH], F32) as hv,
        nc.sbuf_tensor("ev", [L, DSH], F32) as ev,
        nc.sbuf_tensor("tmp", [L, DSH], F32) as tmp,
        nc.sbuf_tensor("pd", [L, 1], F32) as pd,
        nc.sbuf_tensor("dots", [1, L], F32) as dots,
        nc.sbuf_tensor("ea", [1, L], F32) as ea,
        nc.sbuf_tensor("sp", [1, L], F32) as sp,
        nc.sbuf_tensor("lsum", [1, 1], F32) as lsum,
        nc.sbuf_tensor("res", [1, 1], F32) as res,
        nc.sbuf_tensor("warm", [1, 1], F32) as warm,
        nc.semaphore("dma_sem") as dma_sem,
        nc.semaphore("v_sem") as v_sem,
        nc.semaphore("s_sem") as s_sem,
        nc.semaphore("cc_sem") as cc_sem,
        nc.Block() as block,
    ):

        @block.sync
        def _(sync):
            # Row gathers: 64B contiguous per path row, one per descriptor.
            for l, r in enumerate(path):
                sync.dma_start(out=hv[l : l + 1, :], in_=hs[r : r + 1, :]).then_inc(
                    dma_sem, 16
                )
            # Center embedding row, replicated across the L partitions.
            sync.dma_start(
                out=ev[:, :], in_=emd[v_j : v_j + 1, :].broadcast_to([L, DSH])
            ).then_inc(dma_sem, 16)

            # partial dots -> DRAM for the collective
            sync.wait_ge(v_sem, 1)
            sync.dma_start(out=cc_in[0:1, :], in_=pd[0:L, 0:1]).then_inc(dma_sem, 16)

            # reduced dots back to SBUF, free-major
            sync.wait_ge(cc_sem, 1)
            sync.dma_start(out=dots[0:1, :], in_=cc_out[0:1, :]).then_inc(dma_sem, 16)

            # final scalar out
            sync.wait_ge(s_sem, 4)
            sync.dma_start(out=out[:, :], in_=res[:, :]).then_inc(dma_sem, 16)

        @block.vector
        def _(vector):
            # pd[l] = sum_d hv[l,d] * ev[l,d]
            vector.wait_ge(dma_sem, 16 * n_gather)
            vector.scalar_tensor_tensor(
                out=tmp[:, :],
                in0=hv[:, :],
                scalar=1.0,
                in1=ev[:, :],
                op0=mybir.AluOpType.mult,
                op1=mybir.AluOpType.mult,
                accum_out=pd[:, :],
            ).then_inc(v_sem, 1)

        @block.scalar
        def _(scalar):
            # Dummy activation issued before any wait: triggers the ACT
            # table-set load (~2.7us) concurrently with the gather+collective
            # phase instead of on the critical path. Exp and Ln share the
            # `natural_log_exp_and_others` set, so one load covers both.
            scalar.activation(
                warm[:, :],
                nc.const_aps.tensor(0.0, (1, 1)),
                mybir.ActivationFunctionType.Exp,
            ).then_inc(s_sem, 1)

            # sp = softplus(-dots) = log(exp(-dots) + 1) = -log_sigmoid(dots)
            # (this build's ACT tables have no softplus entry; ln+exp live in
            # one table set so this costs pipelined ACT ops, one table load).
            # The product of the L softplus values is exp(sum(ln(sp))): the Ln
            # op's accum_out yields the free-dim sum for free, and one last Exp
            # gives  res = prod(sp) = (-1)^L prod(logsig) = (-1)^(L+1) * answer.
            scalar.wait_ge(dma_sem, 16 * (n_gather + 2))
            scalar.activation(
                ea[:, :],
                dots[:, :],
                mybir.ActivationFunctionType.Exp,
                scale=-1.0,
            ).then_inc(s_sem, 1)
            # ACT pipeline does not forward: same-engine RAW needs waits
            scalar.wait_ge(s_sem, 2)
            scalar.activation(
                sp[:, :],
                ea[:, :],
                mybir.ActivationFunctionType.Ln,
                bias=1.0,
                accum_out=lsum[:, :],
            ).then_inc(s_sem, 1)
            scalar.wait_ge(s_sem, 3)
            scalar.activation(
                res[:, :],
                lsum[:, :],
                mybir.ActivationFunctionType.Exp,
            ).then_inc(s_sem, 1)

        @block.gpsimd
        def _(gpsimd):
            gpsimd.wait_ge(dma_sem, 16 * (n_gather + 1))
            gpsimd.collective_compute(
                "AllReduce",
                mybir.AluOpType.add,
                replica_groups=[list(range(N_CORES))],
                ins=[cc_in[:, :]],
                outs=[cc_out[:, :]],
            ).then_inc(cc_sem, 1)

    # res already equals prod(sp) = (-1)^L * prod(logsig); for odd L the
    # answer -prod(logsig) IS prod(sp). Even L needs a negation on host side
    # of the returned scalar (cheap, exact) — recorded in the sign.
    sign = 1.0 if L % 2 == 1 else -1.0
    return nc, L, sign


_cache: dict = {}


def _get_module(v_j: int, u_k: int):
    key = (v_j, u_k)
    if key not in _cache:
        _cache[key] = build_module(v_j, u_k)
    return _cache[key]


def shard_inputs(emd_np: np.ndarray, hs_np: np.ndarray):
    return [
        {
            "emd": np.ascontiguousarray(emd_np[:, c * DSH : (c + 1) * DSH]),
            "hs": np.ascontiguousarray(hs_np[:, c * DSH : (c + 1) * DSH]),
        }
        for c in range(N_CORES)
    ]


def kernel(v_j, u_k, emd_weight, hs_weight) -> np.ndarray:
    v_j = int(v_j)
    u_k = int(u_k)
    emd_np = np.asarray(emd_weight, dtype=np.float32)
    hs_np = np.asarray(hs_weight, dtype=np.float32)
    assert emd_np.shape == (NUM_V, EMD_DIM), emd_np.shape
    assert hs_np.shape == (NUM_V - 1, EMD_DIM), hs_np.shape

    nc, L, sign = _get_module(v_j, u_k)
    in_maps = shard_inputs(emd_np, hs_np)
    results = run_bass_kernel_spmd(nc, in_maps, list(range(N_CORES))).results
    val = sign * float(results[0]["out"][0, 0])
    return np.float32(val)
